# revision 42
# baseline (speedup 1.0000x reference)
"""Deformable Conv2D (B=8, C=128, H=W=64, ks=3) on 8 Trainium2 cores.

Strategy: data-parallel over batch (1 image per core). Per core:
  1. Offset conv (3x3, pad 1) on PE via 9 PSUM-accumulated matmuls (fp32r).
  2. Transpose offsets to pixel-major; compute bilinear cell (qy, qx) and
     fractions (u, v) on DVE  ->  int16 gather indices + fp32 lerp scalars.
  3. dma_gather (SWDGE) fetches, per tap n and y-corner ky, the contiguous
     (qy+ky, qx..qx+1) channel-pair rows from a padded pixel-major bf16 copy
     of x in HBM.  Output lands pixel-major [pix%128, pix//128, (kx, c)].
  4. Bilinear lerp on DVE with per-partition (per-pixel) scalars:
     two fused scalar_tensor_tensor lerps (kx then ky) per (tile, tap).
  5. PE transposes x_off tiles back to channel-major; main conv = 9
     PSUM-accumulated bf16 matmuls per 512-pixel chunk; bias add on DVE.

Exactness: sample = (1-u)(1-v)x[qy,qx] + (1-u)v x[qy,qx+1] + u(1-v)x[qy+1,qx]
+ uv x[qy+1,qx+1] with qy=floor(py), u=py-qy on the clipped py equals the
reference's 4-corner/overlap formula for every py in [0, 63] (at integer py
the qy+1 row gets weight 0; the padded row/col make the read safe).
"""

import os
import numpy as np
import ml_dtypes
from contextlib import ExitStack

KS = 3
N9 = 9
B, CIN, COUT, H, W = 8, 128, 128, 64, 64
HW = H * W          # 4096
T = HW // 128       # 32 pixel tiles
PW = W + 2          # 66  padded row stride for conv source
PADHW = PW * (H + 2)  # 66*66 = 4356
QI = 32             # parity-table patch grid (i, j in [0, 32))
QROWS = 4 * QI * QI  # 4096 rows, one 2x2x128 patch each
NCORES = 8

_F32 = np.float32
_BF16 = ml_dtypes.bfloat16


# --------------------------------------------------------------------------
# device program
# --------------------------------------------------------------------------

def build_program(half_tiles=2):
    """Emit the single-core program. Returns the finalized Bass object."""
    import concourse.bass as bass
    import concourse.tile as tile
    import concourse.mybir as mybir
    from concourse import bacc

    f32 = mybir.dt.float32
    f32r = mybir.dt.float32r
    bf16 = mybir.dt.bfloat16
    i16 = mybir.dt.int16
    Alu = mybir.AluOpType

    import os
    nc = bacc.Bacc(None, target_bir_lowering=False)
    skip_gather = bool(int(os.environ.get("DK_SKIP_GATHER", "0")))

    # ---- dram parameters (per core) ----
    xP = nc.declare_dram_parameter("xP", [CIN, PADHW], f32, isOutput=False)
    xQ = nc.declare_dram_parameter("xQ", [QROWS, 4 * CIN], bf16, isOutput=False)
    woffT = nc.declare_dram_parameter("woffT", [N9, CIN, 2 * N9], f32, isOutput=False)
    boff = nc.declare_dram_parameter("boff", [2 * N9, 1], f32, isOutput=False)
    wkerT = nc.declare_dram_parameter("wkerT", [N9, CIN, COUT], bf16, isOutput=False)
    bker = nc.declare_dram_parameter("bker", [COUT, 1], f32, isOutput=False)
    basey = nc.declare_dram_parameter("basey", [128, T, N9], f32, isOutput=False)
    basex = nc.declare_dram_parameter("basex", [128, T, N9], f32, isOutput=False)
    ident = nc.declare_dram_parameter("ident", [128, 128], f32, isOutput=False)
    identb = nc.declare_dram_parameter("identb", [128, 128], bf16, isOutput=False)
    out_d = nc.declare_dram_parameter("out", [COUT, HW], f32, isOutput=True)

    with tile.TileContext(nc) as tc, ExitStack() as ctx:
        consts = ctx.enter_context(tc.tile_pool(name="consts", bufs=1))
        maps = ctx.enter_context(tc.tile_pool(name="maps", bufs=1))
        xoffTp = ctx.enter_context(tc.tile_pool(name="xoffT", bufs=1))
        outp = ctx.enter_context(tc.tile_pool(name="outp", bufs=1))
        psA = ctx.enter_context(tc.tile_pool(name="psA", bufs=2, space="PSUM"))
        psB = ctx.enter_context(tc.tile_pool(name="psB", bufs=2, space="PSUM"))
        psF = ctx.enter_context(tc.tile_pool(name="psF", bufs=2, space="PSUM"))
        psG = ctx.enter_context(tc.tile_pool(name="psG", bufs=2, space="PSUM"))

        # ---- load constants ----
        woff_sb = consts.tile([CIN, N9, 2 * N9], f32)
        nc.sync.dma_start(out=woff_sb, in_=woffT[:].rearrange("n k m -> k n m"))
        wker_sb = consts.tile([CIN, N9, COUT], bf16)
        nc.sync.dma_start(out=wker_sb, in_=wkerT[:].rearrange("n k m -> k n m"))
        boff_sb = consts.tile([2 * N9, 1], f32)
        nc.sync.dma_start(out=boff_sb, in_=boff[:])
        bker_sb = consts.tile([COUT, 1], f32)
        nc.sync.dma_start(out=bker_sb, in_=bker[:])
        basey_sb = consts.tile([128, T, N9], f32)
        nc.sync.dma_start(out=basey_sb, in_=basey[:])
        basex_sb = consts.tile([128, T, N9], f32)
        nc.sync.dma_start(out=basex_sb, in_=basex[:])
        ident_sb = consts.tile([128, 128], f32)
        nc.sync.dma_start(out=ident_sb, in_=ident[:])
        identb_sb = consts.tile([128, 128], bf16)
        nc.sync.dma_start(out=identb_sb, in_=identb[:])

        # persistent outputs of the early phases
        otile = maps.tile([128, T, 2 * N9], f32, tag="ot")      # pix-major offsets
        u_sb = maps.tile([128, T, N9], f32, tag="u")            # y fraction
        v_sb = maps.tile([128, T, N9], f32, tag="v")            # x fraction
        idx32 = maps.tile([128, N9, T], mybir.dt.int32, tag="idx32")  # tap-major
        xoffT = xoffTp.tile([CIN, N9, HW], bf16)   # channel-major x_off
        out_sb = outp.tile([COUT, HW], f32)

        # ---------- phase A: offset conv ----------
        with tc.tile_pool(name="loadp", bufs=1) as loadp:
            xP_sb = loadp.tile([CIN, PADHW], f32)
            nc.sync.dma_start(out=xP_sb, in_=xP[:])
            off_sb = loadp.tile([2 * N9, HW], f32)

            # round conv operands to f32r so the fast matmul path verifies
            xPr = loadp.tile([CIN, PADHW], f32r)
            nc.scalar.copy(xPr, xP_sb)
            woffr = consts.tile([CIN, N9, 2 * N9], f32r)
            nc.scalar.copy(woffr, woff_sb)

            xP_v = xPr.rearrange("k (h w) -> k h w", w=PW)
            for ch in range(8):           # 512-pixel chunks = 8 image rows
                ps = psA.tile([2 * N9, 512], f32, tag="psA")
                h0 = ch * 8
                for n in range(N9):
                    dy, dx = n // 3 - 1, n % 3 - 1
                    rhs = xP_v[:, h0 + dy + 1:h0 + dy + 9, dx + 1:dx + 1 + W]
                    nc.tensor.matmul(
                        ps,
                        lhsT=woffr[:, n, :],
                        rhs=rhs,
                        start=(n == 0),
                        stop=(n == N9 - 1),
                    )
                nc.vector.tensor_scalar_add(
                    off_sb[:, ch * 512:(ch + 1) * 512], ps, boff_sb
                )

            # ---------- phase B: transpose offsets to pixel-major ----------
            for g in range(8):            # groups of 4 tiles
                psb = psB.tile([128, 4, 2 * N9], f32, tag="psB")
                for i in range(4):
                    t = g * 4 + i
                    nc.tensor.transpose(
                        psb[:, i, :],
                        off_sb[:, t * 128:(t + 1) * 128],
                        ident_sb[0:2 * N9, 0:2 * N9],
                    )
                nc.scalar.copy(otile[:, g * 4:(g + 1) * 4, :], psb)

        # ---------- phase C: index & fraction math (pixel-major) ----------
        with tc.tile_pool(name="cmaps", bufs=1) as cmaps:
            py = cmaps.tile([128, T, N9], f32, tag="py")
            px = cmaps.tile([128, T, N9], f32, tag="px")
            qy = cmaps.tile([128, T, N9], f32, tag="qy")
            qx = cmaps.tile([128, T, N9], f32, tag="qx")
            idxf = cmaps.tile([128, T, N9], f32, tag="idxf")

            nc.vector.tensor_tensor(py, otile[:, :, 0:N9], basey_sb, op=Alu.add)
            nc.vector.tensor_scalar_min(py, py, float(H - 1))
            nc.vector.tensor_scalar_max(py, py, 0.0)
            nc.vector.tensor_tensor(px, otile[:, :, N9:2 * N9], basex_sb, op=Alu.add)
            nc.vector.tensor_scalar_min(px, px, float(W - 1))
            nc.vector.tensor_scalar_max(px, px, 0.0)
            # floor via int round-trip; fix up round-up cases so u,v in [0,1)
            qi = cmaps.tile([128, T, N9], mybir.dt.int32, tag="qi")
            msk = cmaps.tile([128, T, N9], f32, tag="msk")
            for p_, q_, frac in ((py, qy, u_sb), (px, qx, v_sb)):
                nc.vector.tensor_copy(qi, p_)
                nc.vector.tensor_copy(q_, qi)
                nc.vector.tensor_tensor(frac, p_, q_, op=Alu.subtract)
                nc.vector.tensor_scalar(msk, frac, 0.0, None, op0=Alu.is_lt)
                nc.vector.tensor_tensor(frac, frac, msk, op=Alu.add)
                nc.vector.tensor_tensor(q_, q_, msk, op=Alu.subtract)
            # halve with exact floor: q2 = floor(q / 2), parity = q - 2*q2
            qy2 = cmaps.tile([128, T, N9], f32, tag="qy2")
            qx2 = cmaps.tile([128, T, N9], f32, tag="qx2")
            for q_, q2 in ((qy, qy2), (qx, qx2)):
                nc.vector.tensor_scalar_mul(q2, q_, 0.5)
                nc.vector.tensor_copy(qi, q2)
                nc.vector.tensor_copy(q2, qi)
                # frac = q/2 - q2; fix if negative
                nc.vector.tensor_scalar_mul(msk, q_, 0.5)
                nc.vector.tensor_tensor(msk, msk, q2, op=Alu.subtract)
                nc.vector.tensor_scalar(msk, msk, 0.0, None, op0=Alu.is_lt)
                nc.vector.tensor_tensor(q2, q2, msk, op=Alu.subtract)
            # parity ay = qy - 2*qy2 in {0,1}; idx = (2ay+ax)*1024 + qy2*32 + qx2
            ay = cmaps.tile([128, T, N9], f32, tag="ay")
            ax = cmaps.tile([128, T, N9], f32, tag="ax")
            nc.vector.scalar_tensor_tensor(
                ay, qy2, -2.0, qy, op0=Alu.mult, op1=Alu.add)
            nc.vector.scalar_tensor_tensor(
                ax, qx2, -2.0, qx, op0=Alu.mult, op1=Alu.add)
            nc.vector.scalar_tensor_tensor(
                idxf, qy2, float(QI), qx2, op0=Alu.mult, op1=Alu.add)
            nc.vector.scalar_tensor_tensor(
                idxf, ay, float(2 * QI * QI), idxf, op0=Alu.mult, op1=Alu.add)
            nc.vector.scalar_tensor_tensor(
                idxf, ax, float(QI * QI), idxf, op0=Alu.mult, op1=Alu.add)
            nc.vector.tensor_copy(idx32, idxf.rearrange("p t n -> p n t"))

        # ---------- phases D-F, tiled by pixel halves ----------
        hT = T // half_tiles                       # tiles per half
        with tc.tile_pool(name="work", bufs=2) as work, \
             tc.tile_pool(name="work2", bufs=2) as work2:
            for hf in range(half_tiles):
                t0 = hf * hT
                for n in range(N9):
                    # ---- D: gathers (2 per tap: y-corner 0 / 1) ----
                    g_t = work.tile([128, hT, 4 * CIN], bf16, tag="g")
                    if skip_gather:
                        nc.vector.memset(g_t, 0.0)
                    else:
                        for t in range(hT):
                            nc.gpsimd.indirect_dma_start(
                                out=g_t[:, t, :],
                                out_offset=None,
                                in_=xQ[:],
                                in_offset=bass.IndirectOffsetOnAxis(
                                    ap=idx32[:, n, t0 + t:t0 + t + 1], axis=0
                                ),
                                element_offset=0,
                            )
                    gv = g_t.rearrange("p t (s c) -> p t s c", s=4)

                    # ---- E: bilinear lerp (pixel-major) ----
                    # table rows hold (P00, E=P10-P00, D0=P01-P00, F=..) so
                    # xl = (P00,E) + v*(D0,F) then xoff = xl0 + u*xl1
                    xl = work2.tile([128, hT, 2, CIN], bf16, tag="xl")
                    for t in range(hT):
                        nc.vector.scalar_tensor_tensor(
                            xl[:, t, :, :],
                            gv[:, t, 2:4, :],
                            v_sb[:, t0 + t, n:n + 1],
                            gv[:, t, 0:2, :],
                            op0=Alu.mult,
                            op1=Alu.add,
                        )
                    xoffn = work2.tile([128, hT, CIN], bf16, tag="xoffn")
                    for t in range(hT):
                        nc.vector.scalar_tensor_tensor(
                            xoffn[:, t, :],
                            xl[:, t, 1, :],
                            u_sb[:, t0 + t, n:n + 1],
                            xl[:, t, 0, :],
                            op0=Alu.mult,
                            op1=Alu.add,
                        )

                    # ---- F: transpose x_off back to channel-major ----
                    for g in range(hT // 8):
                        psf = psF.tile([128, 8, 128], bf16, tag="psF")
                        for i in range(8):
                            t = g * 8 + i
                            nc.tensor.transpose(
                                psf[:, i, :], xoffn[:, t, :], identb_sb
                            )
                        dst = xoffT[
                            :, n, (t0 + g * 8) * 128:(t0 + (g + 1) * 8) * 128
                        ].rearrange("k (g c) -> k g c", c=128)
                        nc.scalar.copy(dst, psf)

                # ---- G: main conv over this half ----
                for ch in range(hT * 128 // 512):
                    c0 = t0 * 128 + ch * 512
                    ps = psG.tile([COUT, 512], f32, tag="psG")
                    for n in range(N9):
                        nc.tensor.matmul(
                            ps,
                            lhsT=wker_sb[:, n, :],
                            rhs=xoffT[:, n, c0:c0 + 512],
                            start=(n == 0),
                            stop=(n == N9 - 1),
                        )
                    nc.vector.tensor_scalar_add(out_sb[:, c0:c0 + 512], ps, bker_sb)

        nc.sync.dma_start(out=out_d[:], in_=out_sb)

    nc.compile()
    return nc


# --------------------------------------------------------------------------
# host-side input prep
# --------------------------------------------------------------------------

def _prep_core_inputs(xb, w_off, b_off, w_ker, b_ker):
    """Build the per-core input map from one batch image xb (CIN, H, W)."""
    xb = np.asarray(xb, _F32)

    xP = np.zeros((CIN, H + 2, PW), _F32)
    xP[:, 1:H + 1, 1:W + 1] = xb
    xP = xP.reshape(CIN, PADHW)

    xg = np.zeros((2 * QI + 1, 2 * QI + 1, CIN), _F32)
    xg[:H, :W, :] = np.transpose(xb, (1, 2, 0))
    # parity tables: row (2a+b)*QI*QI + i*QI + j = patch (2i+a:+2, 2j+b:+2)
    P = np.zeros((4, QI, QI, 2, 2, CIN), _F32)
    for a in range(2):
        for b in range(2):
            for ry in range(2):
                for rx in range(2):
                    P[2 * a + b, :, :, ry, rx, :] = xg[
                        a + ry:a + ry + 2 * QI:2, b + rx:b + rx + 2 * QI:2, :]
    # slots: (P00, E=P10-P00, D0=P01-P00, F=P11-P10-P01+P00)
    # xl pair = (P00,E) + v*(D0,F); xoff = xl0 + u*xl1 == bilinear
    xQ = np.zeros((4, QI, QI, 4, CIN), _F32)
    xQ[..., 0, :] = P[..., 0, 0, :]
    xQ[..., 1, :] = P[..., 1, 0, :] - P[..., 0, 0, :]
    xQ[..., 2, :] = P[..., 0, 1, :] - P[..., 0, 0, :]
    xQ[..., 3, :] = P[..., 1, 1, :] - P[..., 1, 0, :] - P[..., 0, 1, :] + P[..., 0, 0, :]
    xQ = xQ.reshape(QROWS, 4 * CIN).astype(_BF16)

    woffT = np.ascontiguousarray(
        np.transpose(w_off.reshape(2 * N9, CIN, N9), (2, 1, 0))
    ).astype(_F32)                                     # [n, cin, 18]
    wkerT = np.ascontiguousarray(
        np.transpose(w_ker.reshape(COUT, CIN, N9), (2, 1, 0))
    ).astype(_BF16)                                    # [n, cin, cout]

    pix = np.arange(HW)
    hh = (pix // W).astype(_F32)
    ww = (pix % W).astype(_F32)
    pny = (np.arange(N9) // 3 - 1).astype(_F32)
    pnx = (np.arange(N9) % 3 - 1).astype(_F32)
    basey = (hh[:, None] + pny[None, :]).reshape(T, 128, N9)
    basex = (ww[:, None] + pnx[None, :]).reshape(T, 128, N9)
    basey = np.ascontiguousarray(np.transpose(basey, (1, 0, 2)))
    basex = np.ascontiguousarray(np.transpose(basex, (1, 0, 2)))

    res = {
        "xP": xP,
        "xQ": xQ,
        "woffT": woffT,
        "boff": np.asarray(b_off, _F32).reshape(2 * N9, 1),
        "wkerT": wkerT,
        "bker": np.asarray(b_ker, _F32).reshape(COUT, 1),
        "basey": basey,
        "basex": basex,
        "ident": np.eye(128, dtype=_F32),
        "identb": np.eye(128, dtype=_BF16),
    }
    return res


_PROGRAM_CACHE = {}


def kernel(x, w_off, b_off, w_ker, b_ker):
    from concourse import bass_utils

    x = np.asarray(x, _F32)
    if "nc" not in _PROGRAM_CACHE:
        _PROGRAM_CACHE["nc"] = build_program()
    nc = _PROGRAM_CACHE["nc"]

    in_maps = [
        _prep_core_inputs(x[b], np.asarray(w_off), np.asarray(b_off),
                          np.asarray(w_ker), np.asarray(b_ker))
        for b in range(B)
    ]
    res = bass_utils.run_bass_kernel_spmd(nc, in_maps, core_ids=list(range(NCORES)))
    global _LAST_RESULTS
    _LAST_RESULTS = res
    out = np.stack([res.results[b]["out"].reshape(COUT, H, W) for b in range(B)])
    return out.astype(np.float32)


_LAST_RESULTS = None


# revision 45
# speedup vs baseline: 1.0522x; 1.0522x over previous
"""Deformable Conv2D (B=8, C=128, H=W=64, ks=3) on 8 Trainium2 cores.

Strategy: data-parallel over batch (1 image per core). Per core:
  1. Offset conv (3x3, pad 1) on PE via 9 PSUM-accumulated matmuls (fp32r).
  2. Transpose offsets to pixel-major; compute bilinear cell (qy, qx) and
     fractions (u, v) on DVE  ->  int16 gather indices + fp32 lerp scalars.
  3. dma_gather (SWDGE) fetches, per tap n and y-corner ky, the contiguous
     (qy+ky, qx..qx+1) channel-pair rows from a padded pixel-major bf16 copy
     of x in HBM.  Output lands pixel-major [pix%128, pix//128, (kx, c)].
  4. Bilinear lerp on DVE with per-partition (per-pixel) scalars:
     two fused scalar_tensor_tensor lerps (kx then ky) per (tile, tap).
  5. PE transposes x_off tiles back to channel-major; main conv = 9
     PSUM-accumulated bf16 matmuls per 512-pixel chunk; bias add on DVE.

Exactness: sample = (1-u)(1-v)x[qy,qx] + (1-u)v x[qy,qx+1] + u(1-v)x[qy+1,qx]
+ uv x[qy+1,qx+1] with qy=floor(py), u=py-qy on the clipped py equals the
reference's 4-corner/overlap formula for every py in [0, 63] (at integer py
the qy+1 row gets weight 0; the padded row/col make the read safe).
"""

import os
import numpy as np
import ml_dtypes
from contextlib import ExitStack

KS = 3
N9 = 9
B, CIN, COUT, H, W = 8, 128, 128, 64, 64
HW = H * W          # 4096
T = HW // 128       # 32 pixel tiles
PW = W + 2          # 66  padded row stride for conv source
PADHW = PW * (H + 2)  # 66*66 = 4356
QI = 32             # parity-table patch grid (i, j in [0, 32))
QROWS = 4 * QI * QI  # 4096 rows, one 2x2x128 patch each
NCORES = 8

_F32 = np.float32
_BF16 = ml_dtypes.bfloat16


# --------------------------------------------------------------------------
# device program
# --------------------------------------------------------------------------

def build_program(half_tiles=4):
    """Emit the single-core program. Returns the finalized Bass object."""
    import concourse.bass as bass
    import concourse.tile as tile
    import concourse.mybir as mybir
    from concourse import bacc

    f32 = mybir.dt.float32
    f32r = mybir.dt.float32r
    bf16 = mybir.dt.bfloat16
    i16 = mybir.dt.int16
    Alu = mybir.AluOpType

    import os
    nc = bacc.Bacc(None, target_bir_lowering=False)
    skip_gather = bool(int(os.environ.get("DK_SKIP_GATHER", "0")))

    # ---- dram parameters (per core) ----
    xP = nc.declare_dram_parameter("xP", [CIN, PADHW], f32r, isOutput=False)
    xQ = nc.declare_dram_parameter("xQ", [QROWS, 4 * CIN], bf16, isOutput=False)
    woffT = nc.declare_dram_parameter("woffT", [N9, CIN, 2 * N9], f32r,
                                       isOutput=False)
    boff = nc.declare_dram_parameter("boff", [2 * N9, 1], f32, isOutput=False)
    wkerT = nc.declare_dram_parameter("wkerT", [N9, CIN, COUT], bf16, isOutput=False)
    bker = nc.declare_dram_parameter("bker", [COUT, 1], f32, isOutput=False)
    basey = nc.declare_dram_parameter("basey", [128, T, N9], f32, isOutput=False)
    basex = nc.declare_dram_parameter("basex", [128, T, N9], f32, isOutput=False)
    ident = nc.declare_dram_parameter("ident", [128, 128], f32, isOutput=False)
    identb = nc.declare_dram_parameter("identb", [128, 128], bf16, isOutput=False)
    out_d = nc.declare_dram_parameter("out", [COUT, HW], f32, isOutput=True)

    with tile.TileContext(nc) as tc, ExitStack() as ctx:
        consts = ctx.enter_context(tc.tile_pool(name="consts", bufs=1))
        maps = ctx.enter_context(tc.tile_pool(name="maps", bufs=1))
        xoffTp = ctx.enter_context(tc.tile_pool(name="xoffT", bufs=1))
        outp = ctx.enter_context(tc.tile_pool(name="outp", bufs=1))
        psA = ctx.enter_context(tc.tile_pool(name="psA", bufs=2, space="PSUM"))
        psB = ctx.enter_context(tc.tile_pool(name="psB", bufs=2, space="PSUM"))
        psF = ctx.enter_context(tc.tile_pool(name="psF", bufs=2, space="PSUM"))
        psG = ctx.enter_context(tc.tile_pool(name="psG", bufs=2, space="PSUM"))

        # ---- load constants ----
        woffr = consts.tile([CIN, N9, 2 * N9], f32r)
        nc.sync.dma_start(out=woffr, in_=woffT[:].rearrange("n k m -> k n m"))
        wker_sb = consts.tile([CIN, N9, COUT], bf16)
        nc.sync.dma_start(out=wker_sb, in_=wkerT[:].rearrange("n k m -> k n m"))
        boff_sb = consts.tile([2 * N9, 1], f32)
        nc.sync.dma_start(out=boff_sb, in_=boff[:])
        bker_sb = consts.tile([COUT, 1], f32)
        nc.sync.dma_start(out=bker_sb, in_=bker[:])
        basey_sb = consts.tile([128, T, N9], f32)
        nc.sync.dma_start(out=basey_sb, in_=basey[:])
        basex_sb = consts.tile([128, T, N9], f32)
        nc.sync.dma_start(out=basex_sb, in_=basex[:])
        ident_sb = consts.tile([128, 128], f32)
        nc.sync.dma_start(out=ident_sb, in_=ident[:])
        identb_sb = consts.tile([128, 128], bf16)
        nc.sync.dma_start(out=identb_sb, in_=identb[:])

        # persistent outputs of the early phases
        otile = maps.tile([128, T, 2 * N9], f32, tag="ot")      # pix-major offsets
        u_sb = maps.tile([128, T, N9], f32, tag="u")            # y fraction
        v_sb = maps.tile([128, T, N9], f32, tag="v")            # x fraction
        idx32 = maps.tile([128, N9, T], mybir.dt.int32, tag="idx32")  # tap-major
        xoffT = xoffTp.tile([CIN, N9, HW], bf16)   # channel-major x_off
        out_sb = outp.tile([COUT, HW], f32)

        # ---------- phase A: offset conv ----------
        with tc.tile_pool(name="loadp", bufs=1) as loadp:
            xPr = loadp.tile([CIN, PADHW], f32r)
            xPr_v = xPr.rearrange("k (h w) -> k h w", w=PW)
            xP_src = xP[:].rearrange("k (h w) -> k h w", w=PW)
            # banded load so the offset conv starts on early rows
            for bd in range(6):
                r0, r1 = bd * 11, min((bd + 1) * 11, H + 2)
                nc.sync.dma_start(out=xPr_v[:, r0:r1, :], in_=xP_src[:, r0:r1, :])
            off_sb = loadp.tile([2 * N9, HW], f32)

            xP_v = xPr_v
            for ch in range(8):           # 512-pixel chunks = 8 image rows
                ps = psA.tile([2 * N9, 512], f32, tag="psA")
                h0 = ch * 8
                for n in range(N9):
                    dy, dx = n // 3 - 1, n % 3 - 1
                    rhs = xP_v[:, h0 + dy + 1:h0 + dy + 9, dx + 1:dx + 1 + W]
                    nc.tensor.matmul(
                        ps,
                        lhsT=woffr[:, n, :],
                        rhs=rhs,
                        start=(n == 0),
                        stop=(n == N9 - 1),
                    )
                nc.vector.tensor_scalar_add(
                    off_sb[:, ch * 512:(ch + 1) * 512], ps, boff_sb
                )

            # ---------- phase B: transpose offsets to pixel-major ----------
            for g in range(8):            # groups of 4 tiles
                psb = psB.tile([128, 4, 2 * N9], f32, tag="psB")
                for i in range(4):
                    t = g * 4 + i
                    nc.tensor.transpose(
                        psb[:, i, :],
                        off_sb[:, t * 128:(t + 1) * 128],
                        ident_sb[0:2 * N9, 0:2 * N9],
                    )
                nc.scalar.copy(otile[:, g * 4:(g + 1) * 4, :], psb)

        # ---------- phase C: index & fraction math (pixel-major) ----------
        with tc.tile_pool(name="cmaps", bufs=1) as cmaps:
            py = cmaps.tile([128, T, N9], f32, tag="py")
            px = cmaps.tile([128, T, N9], f32, tag="px")
            qy = cmaps.tile([128, T, N9], f32, tag="qy")
            qx = cmaps.tile([128, T, N9], f32, tag="qx")
            idxf = cmaps.tile([128, T, N9], f32, tag="idxf")

            nc.vector.tensor_tensor(py, otile[:, :, 0:N9], basey_sb, op=Alu.add)
            nc.vector.tensor_scalar_min(py, py, float(H - 1))
            nc.vector.tensor_scalar_max(py, py, 0.0)
            nc.vector.tensor_tensor(px, otile[:, :, N9:2 * N9], basex_sb, op=Alu.add)
            nc.vector.tensor_scalar_min(px, px, float(W - 1))
            nc.vector.tensor_scalar_max(px, px, 0.0)
            # floor via int round-trip; fix up round-up cases so u,v in [0,1)
            qi = cmaps.tile([128, T, N9], mybir.dt.int32, tag="qi")
            msk = cmaps.tile([128, T, N9], f32, tag="msk")
            for p_, q_, frac in ((py, qy, u_sb), (px, qx, v_sb)):
                nc.vector.tensor_copy(qi, p_)
                nc.vector.tensor_copy(q_, qi)
                nc.vector.tensor_tensor(frac, p_, q_, op=Alu.subtract)
                nc.vector.tensor_scalar(msk, frac, 0.0, None, op0=Alu.is_lt)
                nc.vector.tensor_tensor(frac, frac, msk, op=Alu.add)
                nc.vector.tensor_tensor(q_, q_, msk, op=Alu.subtract)
            # halve with exact floor: q2 = floor(q / 2), parity = q - 2*q2
            qy2 = cmaps.tile([128, T, N9], f32, tag="qy2")
            qx2 = cmaps.tile([128, T, N9], f32, tag="qx2")
            for q_, q2 in ((qy, qy2), (qx, qx2)):
                nc.vector.tensor_scalar_mul(q2, q_, 0.5)
                nc.vector.tensor_copy(qi, q2)
                nc.vector.tensor_copy(q2, qi)
                # frac = q/2 - q2; fix if negative
                nc.vector.tensor_scalar_mul(msk, q_, 0.5)
                nc.vector.tensor_tensor(msk, msk, q2, op=Alu.subtract)
                nc.vector.tensor_scalar(msk, msk, 0.0, None, op0=Alu.is_lt)
                nc.vector.tensor_tensor(q2, q2, msk, op=Alu.subtract)
            # parity ay = qy - 2*qy2 in {0,1}; idx = (2ay+ax)*1024 + qy2*32 + qx2
            ay = cmaps.tile([128, T, N9], f32, tag="ay")
            ax = cmaps.tile([128, T, N9], f32, tag="ax")
            nc.vector.scalar_tensor_tensor(
                ay, qy2, -2.0, qy, op0=Alu.mult, op1=Alu.add)
            nc.vector.scalar_tensor_tensor(
                ax, qx2, -2.0, qx, op0=Alu.mult, op1=Alu.add)
            nc.vector.scalar_tensor_tensor(
                idxf, qy2, float(QI), qx2, op0=Alu.mult, op1=Alu.add)
            nc.vector.scalar_tensor_tensor(
                idxf, ay, float(2 * QI * QI), idxf, op0=Alu.mult, op1=Alu.add)
            nc.vector.scalar_tensor_tensor(
                idxf, ax, float(QI * QI), idxf, op0=Alu.mult, op1=Alu.add)
            nc.vector.tensor_copy(idx32, idxf.rearrange("p t n -> p n t"))

        # ---------- phases D-F, tiled by pixel halves ----------
        hT = T // half_tiles                       # tiles per half
        with tc.tile_pool(name="work", bufs=3) as work, \
             tc.tile_pool(name="work2", bufs=3) as work2:
            for hf in range(half_tiles):
                t0 = hf * hT
                for n in range(N9):
                    # ---- D: gathers (2 per tap: y-corner 0 / 1) ----
                    g_t = work.tile([128, hT, 4 * CIN], bf16, tag="g")
                    if skip_gather:
                        nc.vector.memset(g_t, 0.0)
                    else:
                        for t in range(hT):
                            nc.gpsimd.indirect_dma_start(
                                out=g_t[:, t, :],
                                out_offset=None,
                                in_=xQ[:],
                                in_offset=bass.IndirectOffsetOnAxis(
                                    ap=idx32[:, n, t0 + t:t0 + t + 1], axis=0
                                ),
                                element_offset=0,
                            )
                    gv = g_t.rearrange("p t (s c) -> p t s c", s=4)

                    # ---- E: bilinear lerp (pixel-major) ----
                    # table rows hold (P00, E=P10-P00, D0=P01-P00, F=..) so
                    # xl = (P00,E) + v*(D0,F) then xoff = xl0 + u*xl1
                    xl = work2.tile([128, hT, 2, CIN], bf16, tag="xl")
                    xoffn = work2.tile([128, hT, CIN], bf16, tag="xoffn")
                    if n % 2 == 0:
                        # DVE path: fused lerp via scalar_tensor_tensor
                        for t in range(hT):
                            nc.vector.scalar_tensor_tensor(
                                xl[:, t, :, :],
                                gv[:, t, 2:4, :],
                                v_sb[:, t0 + t, n:n + 1],
                                gv[:, t, 0:2, :],
                                op0=Alu.mult,
                                op1=Alu.add,
                            )
                        for t in range(hT):
                            nc.vector.scalar_tensor_tensor(
                                xoffn[:, t, :],
                                xl[:, t, 1, :],
                                u_sb[:, t0 + t, n:n + 1],
                                xl[:, t, 0, :],
                                op0=Alu.mult,
                                op1=Alu.add,
                            )
                    else:
                        # ACT does the per-pixel-scaled mult, DVE the 2x add
                        Ident = mybir.ActivationFunctionType.Identity
                        for t in range(hT):
                            nc.scalar.activation(
                                xl[:, t, :, :],
                                gv[:, t, 2:4, :],
                                Ident,
                                scale=v_sb[:, t0 + t, n:n + 1],
                            )
                            nc.vector.tensor_tensor(
                                xl[:, t, :, :], xl[:, t, :, :],
                                gv[:, t, 0:2, :], op=Alu.add,
                            )
                        for t in range(hT):
                            nc.scalar.activation(
                                xoffn[:, t, :],
                                xl[:, t, 1, :],
                                Ident,
                                scale=u_sb[:, t0 + t, n:n + 1],
                            )
                            nc.vector.tensor_tensor(
                                xoffn[:, t, :], xoffn[:, t, :],
                                xl[:, t, 0, :], op=Alu.add,
                            )

                    # ---- F: transpose x_off back to channel-major ----
                    for g in range(hT // 8):
                        psf = psF.tile([128, 8, 128], bf16, tag="psF")
                        for i in range(8):
                            t = g * 8 + i
                            nc.tensor.transpose(
                                psf[:, i, :], xoffn[:, t, :], identb_sb
                            )
                        dst = xoffT[
                            :, n, (t0 + g * 8) * 128:(t0 + (g + 1) * 8) * 128
                        ].rearrange("k (g c) -> k g c", c=128)
                        nc.scalar.copy(dst, psf)

                # ---- G: main conv over this half ----
                for ch in range(hT * 128 // 512):
                    c0 = t0 * 128 + ch * 512
                    ps = psG.tile([COUT, 512], f32, tag="psG")
                    for n in range(N9):
                        nc.tensor.matmul(
                            ps,
                            lhsT=wker_sb[:, n, :],
                            rhs=xoffT[:, n, c0:c0 + 512],
                            start=(n == 0),
                            stop=(n == N9 - 1),
                        )
                    nc.scalar.activation(
                        out_sb[:, c0:c0 + 512], ps,
                        mybir.ActivationFunctionType.Identity, bias=bker_sb,
                    )

        nc.sync.dma_start(out=out_d[:], in_=out_sb)

    nc.compile()
    return nc


# --------------------------------------------------------------------------
# host-side input prep
# --------------------------------------------------------------------------

def _prep_core_inputs(xb, w_off, b_off, w_ker, b_ker):
    """Build the per-core input map from one batch image xb (CIN, H, W)."""
    xb = np.asarray(xb, _F32)

    xP = np.zeros((CIN, H + 2, PW), _F32)
    xP[:, 1:H + 1, 1:W + 1] = xb
    xP = xP.reshape(CIN, PADHW)

    xg = np.zeros((2 * QI + 1, 2 * QI + 1, CIN), _F32)
    xg[:H, :W, :] = np.transpose(xb, (1, 2, 0))
    # parity tables: row (2a+b)*QI*QI + i*QI + j = patch (2i+a:+2, 2j+b:+2)
    P = np.zeros((4, QI, QI, 2, 2, CIN), _F32)
    for a in range(2):
        for b in range(2):
            for ry in range(2):
                for rx in range(2):
                    P[2 * a + b, :, :, ry, rx, :] = xg[
                        a + ry:a + ry + 2 * QI:2, b + rx:b + rx + 2 * QI:2, :]
    # slots: (P00, E=P10-P00, D0=P01-P00, F=P11-P10-P01+P00)
    # xl pair = (P00,E) + v*(D0,F); xoff = xl0 + u*xl1 == bilinear
    xQ = np.zeros((4, QI, QI, 4, CIN), _F32)
    xQ[..., 0, :] = P[..., 0, 0, :]
    xQ[..., 1, :] = P[..., 1, 0, :] - P[..., 0, 0, :]
    xQ[..., 2, :] = P[..., 0, 1, :] - P[..., 0, 0, :]
    xQ[..., 3, :] = P[..., 1, 1, :] - P[..., 1, 0, :] - P[..., 0, 1, :] + P[..., 0, 0, :]
    xQ = xQ.reshape(QROWS, 4 * CIN).astype(_BF16)

    woffT = np.ascontiguousarray(
        np.transpose(w_off.reshape(2 * N9, CIN, N9), (2, 1, 0))
    ).astype(_F32)                                     # [n, cin, 18]
    wkerT = np.ascontiguousarray(
        np.transpose(w_ker.reshape(COUT, CIN, N9), (2, 1, 0))
    ).astype(_BF16)                                    # [n, cin, cout]

    pix = np.arange(HW)
    hh = (pix // W).astype(_F32)
    ww = (pix % W).astype(_F32)
    pny = (np.arange(N9) // 3 - 1).astype(_F32)
    pnx = (np.arange(N9) % 3 - 1).astype(_F32)
    basey = (hh[:, None] + pny[None, :]).reshape(T, 128, N9)
    basex = (ww[:, None] + pnx[None, :]).reshape(T, 128, N9)
    basey = np.ascontiguousarray(np.transpose(basey, (1, 0, 2)))
    basex = np.ascontiguousarray(np.transpose(basex, (1, 0, 2)))

    res = {
        "xP": xP,
        "xQ": xQ,
        "woffT": woffT,
        "boff": np.asarray(b_off, _F32).reshape(2 * N9, 1),
        "wkerT": wkerT,
        "bker": np.asarray(b_ker, _F32).reshape(COUT, 1),
        "basey": basey,
        "basex": basex,
        "ident": np.eye(128, dtype=_F32),
        "identb": np.eye(128, dtype=_BF16),
    }
    return res


_PROGRAM_CACHE = {}


def kernel(x, w_off, b_off, w_ker, b_ker):
    from concourse import bass_utils

    x = np.asarray(x, _F32)
    if "nc" not in _PROGRAM_CACHE:
        _PROGRAM_CACHE["nc"] = build_program()
    nc = _PROGRAM_CACHE["nc"]

    in_maps = [
        _prep_core_inputs(x[b], np.asarray(w_off), np.asarray(b_off),
                          np.asarray(w_ker), np.asarray(b_ker))
        for b in range(B)
    ]
    res = bass_utils.run_bass_kernel_spmd(nc, in_maps, core_ids=list(range(NCORES)))
    global _LAST_RESULTS
    _LAST_RESULTS = res
    out = np.stack([res.results[b]["out"].reshape(COUT, H, W) for b in range(B)])
    return out.astype(np.float32)


_LAST_RESULTS = None


# revision 46
# speedup vs baseline: 733.4557x; 697.0893x over previous
"""Deformable Conv2D (B=8, C=128, H=W=64, ks=3) on 8 Trainium2 cores.

Strategy: data-parallel over batch (1 image per core). Per core:
  1. Offset conv (3x3, pad 1) on PE: 9 PSUM-accumulated f32r matmuls per
     512-pixel chunk, on a zero-padded channel-major copy of x.
  2. PE-transpose offsets to pixel-major; DVE computes the bilinear cell
     (qy, qx), fractions (u, v), cell parity, and an int32 patch index.
  3. Gather via indirect_dma_start (one index per partition - the only mode
     real HW supports): each 1KB descriptor fetches one row of a host-built
     parity table xQ holding, per 2x2 patch, the pre-differenced quad
     (P00, P10-P00, P01-P00, P11-P10-P01+P00) in bf16.  Output lands
     pixel-major [pix%128, pix//128, slot, c].
  4. Bilinear lerp with per-partition (per-pixel) scalars, split across
     DVE (fused scalar_tensor_tensor) and ACT+DVE (Identity-scale + add):
     xl pair = (P00,E) + v*(D0,F); xoff = xl0 + u*xl1.
  5. PE transposes x_off back to channel-major; main conv = 9 accumulated
     bf16 matmuls per 512-pixel chunk; bias add on ACT.

Exactness: xoff == (1-u)(1-v)P00 + (1-u)v P01 + u(1-v)P10 + uv P11 with
qy=floor(py), u=py-qy on the clipped py equals the reference's 4-corner /
overlap formula for every py in [0, 63] (at integer py the qy+1 row gets
weight 0; padded rows/cols make those reads safe).  bf16 table + matmuls
give ~5e-3 max relative error vs the fp32 reference.
"""

import os
import numpy as np
import ml_dtypes
from contextlib import ExitStack

KS = 3
N9 = 9
B, CIN, COUT, H, W = 8, 128, 128, 64, 64
HW = H * W          # 4096
T = HW // 128       # 32 pixel tiles
PW = W + 2          # 66  padded row stride for conv source
PADHW = PW * (H + 2)  # 66*66 = 4356
QI = 32             # parity-table patch grid (i, j in [0, 32))
QROWS = 4 * QI * QI  # 4096 rows, one 2x2x128 patch each
NCORES = 8

_F32 = np.float32
_BF16 = ml_dtypes.bfloat16


# --------------------------------------------------------------------------
# device program
# --------------------------------------------------------------------------

def build_program(half_tiles=4):
    """Emit the single-core program. Returns the finalized Bass object."""
    import concourse.bass as bass
    import concourse.tile as tile
    import concourse.mybir as mybir
    from concourse import bacc

    f32 = mybir.dt.float32
    f32r = mybir.dt.float32r
    bf16 = mybir.dt.bfloat16
    i16 = mybir.dt.int16
    Alu = mybir.AluOpType

    import os
    nc = bacc.Bacc(None, target_bir_lowering=False)
    skip_gather = bool(int(os.environ.get("DK_SKIP_GATHER", "0")))

    # ---- dram parameters (per core) ----
    xP = nc.declare_dram_parameter("xP", [CIN, PADHW], f32r, isOutput=False)
    xQ = nc.declare_dram_parameter("xQ", [QROWS, 4 * CIN], bf16, isOutput=False)
    woffT = nc.declare_dram_parameter("woffT", [N9, CIN, 2 * N9], f32r,
                                       isOutput=False)
    boff = nc.declare_dram_parameter("boff", [2 * N9, 1], f32, isOutput=False)
    wkerT = nc.declare_dram_parameter("wkerT", [N9, CIN, COUT], bf16, isOutput=False)
    bker = nc.declare_dram_parameter("bker", [COUT, 1], f32, isOutput=False)
    basey = nc.declare_dram_parameter("basey", [128, T, N9], f32, isOutput=False)
    basex = nc.declare_dram_parameter("basex", [128, T, N9], f32, isOutput=False)
    ident = nc.declare_dram_parameter("ident", [128, 128], f32, isOutput=False)
    identb = nc.declare_dram_parameter("identb", [128, 128], bf16, isOutput=False)
    out_d = nc.declare_dram_parameter("out", [COUT, HW], f32, isOutput=True)

    with tile.TileContext(nc) as tc, ExitStack() as ctx:
        consts = ctx.enter_context(tc.tile_pool(name="consts", bufs=1))
        maps = ctx.enter_context(tc.tile_pool(name="maps", bufs=1))
        xoffTp = ctx.enter_context(tc.tile_pool(name="xoffT", bufs=1))
        outp = ctx.enter_context(tc.tile_pool(name="outp", bufs=1))
        psA = ctx.enter_context(tc.tile_pool(name="psA", bufs=2, space="PSUM"))
        psB = ctx.enter_context(tc.tile_pool(name="psB", bufs=2, space="PSUM"))
        psF = ctx.enter_context(tc.tile_pool(name="psF", bufs=2, space="PSUM"))
        psG = ctx.enter_context(tc.tile_pool(name="psG", bufs=2, space="PSUM"))

        # ---- load constants ----
        woffr = consts.tile([CIN, N9, 2 * N9], f32r)
        nc.sync.dma_start(out=woffr, in_=woffT[:].rearrange("n k m -> k n m"))
        wker_sb = consts.tile([CIN, N9, COUT], bf16)
        nc.sync.dma_start(out=wker_sb, in_=wkerT[:].rearrange("n k m -> k n m"))
        boff_sb = consts.tile([2 * N9, 1], f32)
        nc.sync.dma_start(out=boff_sb, in_=boff[:])
        bker_sb = consts.tile([COUT, 1], f32)
        nc.sync.dma_start(out=bker_sb, in_=bker[:])
        basey_sb = consts.tile([128, T, N9], f32)
        nc.sync.dma_start(out=basey_sb, in_=basey[:])
        basex_sb = consts.tile([128, T, N9], f32)
        nc.sync.dma_start(out=basex_sb, in_=basex[:])
        ident_sb = consts.tile([128, 128], f32)
        nc.sync.dma_start(out=ident_sb, in_=ident[:])
        identb_sb = consts.tile([128, 128], bf16)
        nc.sync.dma_start(out=identb_sb, in_=identb[:])

        # persistent outputs of the early phases
        otile = maps.tile([128, T, 2 * N9], f32, tag="ot")      # pix-major offsets
        u_sb = maps.tile([128, T, N9], f32, tag="u")            # y fraction
        v_sb = maps.tile([128, T, N9], f32, tag="v")            # x fraction
        idx32 = maps.tile([128, N9, T], mybir.dt.int32, tag="idx32")  # tap-major
        xoffT = xoffTp.tile([CIN, N9, HW], bf16)   # channel-major x_off
        out_sb = outp.tile([COUT, HW], f32)

        # ---------- phase A: offset conv ----------
        with tc.tile_pool(name="loadp", bufs=1) as loadp:
            xPr = loadp.tile([CIN, PADHW], f32r)
            xPr_v = xPr.rearrange("k (h w) -> k h w", w=PW)
            xP_src = xP[:].rearrange("k (h w) -> k h w", w=PW)
            # banded load so the offset conv starts on early rows
            for bd in range(6):
                r0, r1 = bd * 11, min((bd + 1) * 11, H + 2)
                nc.sync.dma_start(out=xPr_v[:, r0:r1, :], in_=xP_src[:, r0:r1, :])
            off_sb = loadp.tile([2 * N9, HW], f32)

            xP_v = xPr_v
            for ch in range(8):           # 512-pixel chunks = 8 image rows
                ps = psA.tile([2 * N9, 512], f32, tag="psA")
                h0 = ch * 8
                for n in range(N9):
                    dy, dx = n // 3 - 1, n % 3 - 1
                    rhs = xP_v[:, h0 + dy + 1:h0 + dy + 9, dx + 1:dx + 1 + W]
                    nc.tensor.matmul(
                        ps,
                        lhsT=woffr[:, n, :],
                        rhs=rhs,
                        start=(n == 0),
                        stop=(n == N9 - 1),
                    )
                nc.vector.tensor_scalar_add(
                    off_sb[:, ch * 512:(ch + 1) * 512], ps, boff_sb
                )

            # ---------- phase B: transpose offsets to pixel-major ----------
            for g in range(8):            # groups of 4 tiles
                psb = psB.tile([128, 4, 2 * N9], f32, tag="psB")
                for i in range(4):
                    t = g * 4 + i
                    nc.tensor.transpose(
                        psb[:, i, :],
                        off_sb[:, t * 128:(t + 1) * 128],
                        ident_sb[0:2 * N9, 0:2 * N9],
                    )
                nc.scalar.copy(otile[:, g * 4:(g + 1) * 4, :], psb)

        # ---------- phase C: index & fraction math (pixel-major) ----------
        with tc.tile_pool(name="cmaps", bufs=1) as cmaps:
            py = cmaps.tile([128, T, N9], f32, tag="py")
            px = cmaps.tile([128, T, N9], f32, tag="px")
            qy = cmaps.tile([128, T, N9], f32, tag="qy")
            qx = cmaps.tile([128, T, N9], f32, tag="qx")
            idxf = cmaps.tile([128, T, N9], f32, tag="idxf")

            nc.vector.tensor_tensor(py, otile[:, :, 0:N9], basey_sb, op=Alu.add)
            nc.vector.tensor_scalar_min(py, py, float(H - 1))
            nc.vector.tensor_scalar_max(py, py, 0.0)
            nc.vector.tensor_tensor(px, otile[:, :, N9:2 * N9], basex_sb, op=Alu.add)
            nc.vector.tensor_scalar_min(px, px, float(W - 1))
            nc.vector.tensor_scalar_max(px, px, 0.0)
            # floor via int round-trip; fix up round-up cases so u,v in [0,1)
            qi = cmaps.tile([128, T, N9], mybir.dt.int32, tag="qi")
            msk = cmaps.tile([128, T, N9], f32, tag="msk")
            for p_, q_, frac in ((py, qy, u_sb), (px, qx, v_sb)):
                nc.vector.tensor_copy(qi, p_)
                nc.vector.tensor_copy(q_, qi)
                nc.vector.tensor_tensor(frac, p_, q_, op=Alu.subtract)
                nc.vector.tensor_scalar(msk, frac, 0.0, None, op0=Alu.is_lt)
                nc.vector.tensor_tensor(frac, frac, msk, op=Alu.add)
                nc.vector.tensor_tensor(q_, q_, msk, op=Alu.subtract)
            # halve with exact floor: q2 = floor(q / 2), parity = q - 2*q2
            qy2 = cmaps.tile([128, T, N9], f32, tag="qy2")
            qx2 = cmaps.tile([128, T, N9], f32, tag="qx2")
            for q_, q2 in ((qy, qy2), (qx, qx2)):
                nc.vector.tensor_scalar_mul(q2, q_, 0.5)
                nc.vector.tensor_copy(qi, q2)
                nc.vector.tensor_copy(q2, qi)
                # frac = q/2 - q2; fix if negative
                nc.vector.tensor_scalar_mul(msk, q_, 0.5)
                nc.vector.tensor_tensor(msk, msk, q2, op=Alu.subtract)
                nc.vector.tensor_scalar(msk, msk, 0.0, None, op0=Alu.is_lt)
                nc.vector.tensor_tensor(q2, q2, msk, op=Alu.subtract)
            # parity ay = qy - 2*qy2 in {0,1}; idx = (2ay+ax)*1024 + qy2*32 + qx2
            ay = cmaps.tile([128, T, N9], f32, tag="ay")
            ax = cmaps.tile([128, T, N9], f32, tag="ax")
            nc.vector.scalar_tensor_tensor(
                ay, qy2, -2.0, qy, op0=Alu.mult, op1=Alu.add)
            nc.vector.scalar_tensor_tensor(
                ax, qx2, -2.0, qx, op0=Alu.mult, op1=Alu.add)
            nc.vector.scalar_tensor_tensor(
                idxf, qy2, float(QI), qx2, op0=Alu.mult, op1=Alu.add)
            nc.vector.scalar_tensor_tensor(
                idxf, ay, float(2 * QI * QI), idxf, op0=Alu.mult, op1=Alu.add)
            nc.vector.scalar_tensor_tensor(
                idxf, ax, float(QI * QI), idxf, op0=Alu.mult, op1=Alu.add)
            nc.vector.tensor_copy(idx32, idxf.rearrange("p t n -> p n t"))

        # ---------- phases D-F, tiled by pixel halves ----------
        hT = T // half_tiles                       # tiles per half
        with tc.tile_pool(name="work", bufs=3) as work, \
             tc.tile_pool(name="work2", bufs=3) as work2:
            for hf in range(half_tiles):
                t0 = hf * hT
                for n in range(N9):
                    # ---- D: gathers (2 per tap: y-corner 0 / 1) ----
                    g_t = work.tile([128, hT, 4 * CIN], bf16, tag="g")
                    if skip_gather:
                        nc.vector.memset(g_t, 0.0)
                    else:
                        for t in range(hT):
                            nc.gpsimd.indirect_dma_start(
                                out=g_t[:, t, :],
                                out_offset=None,
                                in_=xQ[:],
                                in_offset=bass.IndirectOffsetOnAxis(
                                    ap=idx32[:, n, t0 + t:t0 + t + 1], axis=0
                                ),
                                element_offset=0,
                            )
                    gv = g_t.rearrange("p t (s c) -> p t s c", s=4)

                    # ---- E: bilinear lerp (pixel-major) ----
                    # table rows hold (P00, E=P10-P00, D0=P01-P00, F=..) so
                    # xl = (P00,E) + v*(D0,F) then xoff = xl0 + u*xl1
                    xl = work2.tile([128, hT, 2, CIN], bf16, tag="xl")
                    xoffn = work2.tile([128, hT, CIN], bf16, tag="xoffn")
                    if n % 2 == 0:
                        # DVE path: fused lerp via scalar_tensor_tensor
                        for t in range(hT):
                            nc.vector.scalar_tensor_tensor(
                                xl[:, t, :, :],
                                gv[:, t, 2:4, :],
                                v_sb[:, t0 + t, n:n + 1],
                                gv[:, t, 0:2, :],
                                op0=Alu.mult,
                                op1=Alu.add,
                            )
                        for t in range(hT):
                            nc.vector.scalar_tensor_tensor(
                                xoffn[:, t, :],
                                xl[:, t, 1, :],
                                u_sb[:, t0 + t, n:n + 1],
                                xl[:, t, 0, :],
                                op0=Alu.mult,
                                op1=Alu.add,
                            )
                    else:
                        # ACT does the per-pixel-scaled mult, DVE the 2x add
                        Ident = mybir.ActivationFunctionType.Identity
                        for t in range(hT):
                            nc.scalar.activation(
                                xl[:, t, :, :],
                                gv[:, t, 2:4, :],
                                Ident,
                                scale=v_sb[:, t0 + t, n:n + 1],
                            )
                            nc.vector.tensor_tensor(
                                xl[:, t, :, :], xl[:, t, :, :],
                                gv[:, t, 0:2, :], op=Alu.add,
                            )
                        for t in range(hT):
                            nc.scalar.activation(
                                xoffn[:, t, :],
                                xl[:, t, 1, :],
                                Ident,
                                scale=u_sb[:, t0 + t, n:n + 1],
                            )
                            nc.vector.tensor_tensor(
                                xoffn[:, t, :], xoffn[:, t, :],
                                xl[:, t, 0, :], op=Alu.add,
                            )

                    # ---- F: transpose x_off back to channel-major ----
                    for g in range(hT // 8):
                        psf = psF.tile([128, 8, 128], bf16, tag="psF")
                        for i in range(8):
                            t = g * 8 + i
                            nc.tensor.transpose(
                                psf[:, i, :], xoffn[:, t, :], identb_sb
                            )
                        dst = xoffT[
                            :, n, (t0 + g * 8) * 128:(t0 + (g + 1) * 8) * 128
                        ].rearrange("k (g c) -> k g c", c=128)
                        nc.scalar.copy(dst, psf)

                # ---- G: main conv over this half ----
                for ch in range(hT * 128 // 512):
                    c0 = t0 * 128 + ch * 512
                    ps = psG.tile([COUT, 512], f32, tag="psG")
                    for n in range(N9):
                        nc.tensor.matmul(
                            ps,
                            lhsT=wker_sb[:, n, :],
                            rhs=xoffT[:, n, c0:c0 + 512],
                            start=(n == 0),
                            stop=(n == N9 - 1),
                        )
                    nc.scalar.activation(
                        out_sb[:, c0:c0 + 512], ps,
                        mybir.ActivationFunctionType.Identity, bias=bker_sb,
                    )

        nc.sync.dma_start(out=out_d[:], in_=out_sb)

    nc.compile()
    return nc


# --------------------------------------------------------------------------
# host-side input prep
# --------------------------------------------------------------------------

def _prep_core_inputs(xb, w_off, b_off, w_ker, b_ker):
    """Build the per-core input map from one batch image xb (CIN, H, W)."""
    xb = np.asarray(xb, _F32)

    xP = np.zeros((CIN, H + 2, PW), _F32)
    xP[:, 1:H + 1, 1:W + 1] = xb
    xP = xP.reshape(CIN, PADHW)

    xg = np.zeros((2 * QI + 1, 2 * QI + 1, CIN), _F32)
    xg[:H, :W, :] = np.transpose(xb, (1, 2, 0))
    # parity tables: row (2a+b)*QI*QI + i*QI + j = patch (2i+a:+2, 2j+b:+2)
    P = np.zeros((4, QI, QI, 2, 2, CIN), _F32)
    for a in range(2):
        for b in range(2):
            for ry in range(2):
                for rx in range(2):
                    P[2 * a + b, :, :, ry, rx, :] = xg[
                        a + ry:a + ry + 2 * QI:2, b + rx:b + rx + 2 * QI:2, :]
    # slots: (P00, E=P10-P00, D0=P01-P00, F=P11-P10-P01+P00)
    # xl pair = (P00,E) + v*(D0,F); xoff = xl0 + u*xl1 == bilinear
    xQ = np.zeros((4, QI, QI, 4, CIN), _F32)
    xQ[..., 0, :] = P[..., 0, 0, :]
    xQ[..., 1, :] = P[..., 1, 0, :] - P[..., 0, 0, :]
    xQ[..., 2, :] = P[..., 0, 1, :] - P[..., 0, 0, :]
    xQ[..., 3, :] = P[..., 1, 1, :] - P[..., 1, 0, :] - P[..., 0, 1, :] + P[..., 0, 0, :]
    xQ = xQ.reshape(QROWS, 4 * CIN).astype(_BF16)

    woffT = np.ascontiguousarray(
        np.transpose(w_off.reshape(2 * N9, CIN, N9), (2, 1, 0))
    ).astype(_F32)                                     # [n, cin, 18]
    wkerT = np.ascontiguousarray(
        np.transpose(w_ker.reshape(COUT, CIN, N9), (2, 1, 0))
    ).astype(_BF16)                                    # [n, cin, cout]

    pix = np.arange(HW)
    hh = (pix // W).astype(_F32)
    ww = (pix % W).astype(_F32)
    pny = (np.arange(N9) // 3 - 1).astype(_F32)
    pnx = (np.arange(N9) % 3 - 1).astype(_F32)
    basey = (hh[:, None] + pny[None, :]).reshape(T, 128, N9)
    basex = (ww[:, None] + pnx[None, :]).reshape(T, 128, N9)
    basey = np.ascontiguousarray(np.transpose(basey, (1, 0, 2)))
    basex = np.ascontiguousarray(np.transpose(basex, (1, 0, 2)))

    res = {
        "xP": xP,
        "xQ": xQ,
        "woffT": woffT,
        "boff": np.asarray(b_off, _F32).reshape(2 * N9, 1),
        "wkerT": wkerT,
        "bker": np.asarray(b_ker, _F32).reshape(COUT, 1),
        "basey": basey,
        "basex": basex,
        "ident": np.eye(128, dtype=_F32),
        "identb": np.eye(128, dtype=_BF16),
    }
    return res


_PROGRAM_CACHE = {}


def kernel(x, w_off, b_off, w_ker, b_ker):
    from concourse import bass_utils

    x = np.asarray(x, _F32)
    if "nc" not in _PROGRAM_CACHE:
        _PROGRAM_CACHE["nc"] = build_program()
    nc = _PROGRAM_CACHE["nc"]

    in_maps = [
        _prep_core_inputs(x[b], np.asarray(w_off), np.asarray(b_off),
                          np.asarray(w_ker), np.asarray(b_ker))
        for b in range(B)
    ]
    res = bass_utils.run_bass_kernel_spmd(nc, in_maps, core_ids=list(range(NCORES)))
    global _LAST_RESULTS
    _LAST_RESULTS = res
    out = np.stack([res.results[b]["out"].reshape(COUT, H, W) for b in range(B)])
    return out.astype(np.float32)


_LAST_RESULTS = None


# revision 50
# speedup vs baseline: 1186.0627x; 1.6171x over previous
"""Deformable Conv2D (B=8, C=128, H=W=64, ks=3) on 8 Trainium2 cores.

Strategy: data-parallel over batch (1 image per core). Per core:
  1. Offset conv (3x3, pad 1) on PE: 9 PSUM-accumulated f32r matmuls per
     512-pixel chunk, on a zero-padded channel-major copy of x.
  2. PE-transpose offsets to pixel-major; DVE computes the bilinear cell
     (qy, qx), fractions (u, v), cell parity, and an int32 patch index.
  3. Gather via indirect_dma_start (one index per partition - the only mode
     real HW supports): each 1KB descriptor fetches one row of a host-built
     parity table xQ holding, per 2x2 patch, the pre-differenced quad
     (P00, P10-P00, P01-P00, P11-P10-P01+P00) in bf16.  Output lands
     pixel-major [pix%128, pix//128, slot, c].
  4. Bilinear lerp with per-partition (per-pixel) scalars, split across
     DVE (fused scalar_tensor_tensor) and ACT+DVE (Identity-scale + add):
     xl pair = (P00,E) + v*(D0,F); xoff = xl0 + u*xl1.
  5. PE transposes x_off back to channel-major; main conv = 9 accumulated
     bf16 matmuls per 512-pixel chunk; bias add on ACT.

Exactness: xoff == (1-u)(1-v)P00 + (1-u)v P01 + u(1-v)P10 + uv P11 with
qy=floor(py), u=py-qy on the clipped py equals the reference's 4-corner /
overlap formula for every py in [0, 63] (at integer py the qy+1 row gets
weight 0; padded rows/cols make those reads safe).  bf16 table + matmuls
give ~5e-3 max relative error vs the fp32 reference.
"""

import os
import sys
import numpy as np
import ml_dtypes
from contextlib import ExitStack

for _p in ("/opt/trn_rl_repo",):
    if os.path.isdir(_p) and _p not in sys.path:
        sys.path.insert(0, _p)

KS = 3
N9 = 9
B, CIN, COUT, H, W = 8, 128, 128, 64, 64
HW = H * W          # 4096
T = HW // 128       # 32 pixel tiles
PW = W + 2          # 66  padded row stride for conv source
PADHW = PW * (H + 2)  # 66*66 = 4356
QI = 32             # parity-table patch grid (i, j in [0, 32))
QROWS = 4 * QI * QI  # 4096 rows, one 2x2x128 patch each
NCORES = 8

_F32 = np.float32
_BF16 = ml_dtypes.bfloat16


# --------------------------------------------------------------------------
# device program
# --------------------------------------------------------------------------

def build_program(half_tiles=None):
    if half_tiles is None:
        half_tiles = int(os.environ.get("DK_HALVES", "4"))
    """Emit the single-core program. Returns the finalized Bass object."""
    import concourse.bass as bass
    import concourse.tile as tile
    import concourse.mybir as mybir
    from concourse import bacc

    f32 = mybir.dt.float32
    f32r = mybir.dt.float32r
    bf16 = mybir.dt.bfloat16
    i16 = mybir.dt.int16
    Alu = mybir.AluOpType

    nc = bacc.Bacc(None, target_bir_lowering=False)
    skip_gather = bool(int(os.environ.get("DK_SKIP_GATHER", "0")))

    # ---- dram parameters (per core) ----
    xP = nc.declare_dram_parameter("xP", [CIN, PADHW], f32r, isOutput=False)
    xQ = nc.declare_dram_parameter("xQ", [QROWS, 4 * CIN], bf16, isOutput=False)
    woffT = nc.declare_dram_parameter("woffT", [N9, CIN, 2 * N9], f32r,
                                       isOutput=False)
    boff = nc.declare_dram_parameter("boff", [2 * N9, 1], f32, isOutput=False)
    wkerT = nc.declare_dram_parameter("wkerT", [N9, CIN, COUT], bf16, isOutput=False)
    bker = nc.declare_dram_parameter("bker", [COUT, 1], f32, isOutput=False)
    basey = nc.declare_dram_parameter("basey", [128, T, N9], f32, isOutput=False)
    basex = nc.declare_dram_parameter("basex", [128, T, N9], f32, isOutput=False)
    ident = nc.declare_dram_parameter("ident", [128, 128], f32, isOutput=False)
    identb = nc.declare_dram_parameter("identb", [128, 128], bf16, isOutput=False)
    out_d = nc.declare_dram_parameter("out", [COUT, HW], f32, isOutput=True)

    with tile.TileContext(nc) as tc, ExitStack() as ctx:
        consts = ctx.enter_context(tc.tile_pool(name="consts", bufs=1))
        maps = ctx.enter_context(tc.tile_pool(name="maps", bufs=1))
        xoffTp = ctx.enter_context(tc.tile_pool(name="xoffT", bufs=1))
        outp = ctx.enter_context(tc.tile_pool(name="outp", bufs=1))
        psA = ctx.enter_context(tc.tile_pool(name="psA", bufs=2, space="PSUM"))
        psB = ctx.enter_context(tc.tile_pool(name="psB", bufs=2, space="PSUM"))
        psF = ctx.enter_context(tc.tile_pool(name="psF", bufs=2, space="PSUM"))
        psG = ctx.enter_context(tc.tile_pool(name="psG", bufs=2, space="PSUM"))

        # ---- load constants ----
        woffr = consts.tile([CIN, N9, 2 * N9], f32r)
        nc.sync.dma_start(out=woffr, in_=woffT[:].rearrange("n k m -> k n m"))
        wker_sb = consts.tile([CIN, N9, COUT], bf16)
        nc.sync.dma_start(out=wker_sb, in_=wkerT[:].rearrange("n k m -> k n m"))
        boff_sb = consts.tile([2 * N9, 1], f32)
        nc.sync.dma_start(out=boff_sb, in_=boff[:])
        bker_sb = consts.tile([COUT, 1], f32)
        nc.sync.dma_start(out=bker_sb, in_=bker[:])
        basey_sb = consts.tile([128, T, N9], f32)
        nc.sync.dma_start(out=basey_sb, in_=basey[:])
        basex_sb = consts.tile([128, T, N9], f32)
        nc.sync.dma_start(out=basex_sb, in_=basex[:])
        ident_sb = consts.tile([128, 128], f32)
        nc.sync.dma_start(out=ident_sb, in_=ident[:])
        identb_sb = consts.tile([128, 128], bf16)
        nc.sync.dma_start(out=identb_sb, in_=identb[:])

        # persistent outputs of the early phases
        otile = maps.tile([128, T, 2 * N9], f32, tag="ot")      # pix-major offsets
        u_sb = maps.tile([128, T, N9], f32, tag="u")            # y fraction
        v_sb = maps.tile([128, T, N9], f32, tag="v")            # x fraction
        idx32 = maps.tile([128, N9, T], mybir.dt.int32, tag="idx32")  # tap-major
        xoffT = xoffTp.tile([CIN, N9, HW], bf16)   # channel-major x_off
        out_sb = outp.tile([COUT, HW], f32)

        # ---------- phase A: offset conv ----------
        with tc.tile_pool(name="loadp", bufs=1) as loadp:
            xPr = loadp.tile([CIN, PADHW], f32r)
            xPr_v = xPr.rearrange("k (h w) -> k h w", w=PW)
            xP_src = xP[:].rearrange("k (h w) -> k h w", w=PW)
            # banded load so the offset conv starts on early rows
            for bd in range(6):
                r0, r1 = bd * 11, min((bd + 1) * 11, H + 2)
                nc.sync.dma_start(out=xPr_v[:, r0:r1, :], in_=xP_src[:, r0:r1, :])
            off_sb = loadp.tile([2 * N9, HW], f32)

            xP_v = xPr_v
            for ch in range(8):           # 512-pixel chunks = 8 image rows
                ps = psA.tile([2 * N9, 512], f32, tag="psA")
                h0 = ch * 8
                for n in range(N9):
                    dy, dx = n // 3 - 1, n % 3 - 1
                    rhs = xP_v[:, h0 + dy + 1:h0 + dy + 9, dx + 1:dx + 1 + W]
                    nc.tensor.matmul(
                        ps,
                        lhsT=woffr[:, n, :],
                        rhs=rhs,
                        start=(n == 0),
                        stop=(n == N9 - 1),
                    )
                nc.vector.tensor_scalar_add(
                    off_sb[:, ch * 512:(ch + 1) * 512], ps, boff_sb
                )

            # ---------- phase B: transpose offsets to pixel-major ----------
            for g in range(8):            # groups of 4 tiles
                psb = psB.tile([128, 4, 2 * N9], f32, tag="psB")
                for i in range(4):
                    t = g * 4 + i
                    nc.tensor.transpose(
                        psb[:, i, :],
                        off_sb[:, t * 128:(t + 1) * 128],
                        ident_sb[0:2 * N9, 0:2 * N9],
                    )
                nc.scalar.copy(otile[:, g * 4:(g + 1) * 4, :], psb)

        # ---------- phase C: index & fraction math (pixel-major) ----------
        with tc.tile_pool(name="cmaps", bufs=1) as cmaps:
            py = cmaps.tile([128, T, N9], f32, tag="py")
            px = cmaps.tile([128, T, N9], f32, tag="px")
            qy = cmaps.tile([128, T, N9], f32, tag="qy")
            qx = cmaps.tile([128, T, N9], f32, tag="qx")
            idxf = cmaps.tile([128, T, N9], f32, tag="idxf")

            nc.vector.tensor_tensor(py, otile[:, :, 0:N9], basey_sb, op=Alu.add)
            nc.vector.tensor_scalar_min(py, py, float(H - 1))
            nc.vector.tensor_scalar_max(py, py, 0.0)
            nc.vector.tensor_tensor(px, otile[:, :, N9:2 * N9], basex_sb, op=Alu.add)
            nc.vector.tensor_scalar_min(px, px, float(W - 1))
            nc.vector.tensor_scalar_max(px, px, 0.0)
            # floor via int round-trip; fix up round-up cases so u,v in [0,1)
            qi = cmaps.tile([128, T, N9], mybir.dt.int32, tag="qi")
            msk = cmaps.tile([128, T, N9], f32, tag="msk")
            for p_, q_, frac in ((py, qy, u_sb), (px, qx, v_sb)):
                nc.vector.tensor_copy(qi, p_)
                nc.vector.tensor_copy(q_, qi)
                nc.vector.tensor_tensor(frac, p_, q_, op=Alu.subtract)
                nc.vector.tensor_scalar(msk, frac, 0.0, None, op0=Alu.is_lt)
                nc.vector.tensor_tensor(frac, frac, msk, op=Alu.add)
                nc.vector.tensor_tensor(q_, q_, msk, op=Alu.subtract)
            # halve with exact floor: q2 = floor(q / 2), parity = q - 2*q2
            qy2 = cmaps.tile([128, T, N9], f32, tag="qy2")
            qx2 = cmaps.tile([128, T, N9], f32, tag="qx2")
            for q_, q2 in ((qy, qy2), (qx, qx2)):
                nc.vector.tensor_scalar_mul(q2, q_, 0.5)
                nc.vector.tensor_copy(qi, q2)
                nc.vector.tensor_copy(q2, qi)
                # frac = q/2 - q2; fix if negative
                nc.vector.tensor_scalar_mul(msk, q_, 0.5)
                nc.vector.tensor_tensor(msk, msk, q2, op=Alu.subtract)
                nc.vector.tensor_scalar(msk, msk, 0.0, None, op0=Alu.is_lt)
                nc.vector.tensor_tensor(q2, q2, msk, op=Alu.subtract)
            # parity ay = qy - 2*qy2 in {0,1}; idx = (2ay+ax)*1024 + qy2*32 + qx2
            ay = cmaps.tile([128, T, N9], f32, tag="ay")
            ax = cmaps.tile([128, T, N9], f32, tag="ax")
            nc.vector.scalar_tensor_tensor(
                ay, qy2, -2.0, qy, op0=Alu.mult, op1=Alu.add)
            nc.vector.scalar_tensor_tensor(
                ax, qx2, -2.0, qx, op0=Alu.mult, op1=Alu.add)
            nc.vector.scalar_tensor_tensor(
                idxf, qy2, float(QI), qx2, op0=Alu.mult, op1=Alu.add)
            nc.vector.scalar_tensor_tensor(
                idxf, ay, float(2 * QI * QI), idxf, op0=Alu.mult, op1=Alu.add)
            nc.vector.scalar_tensor_tensor(
                idxf, ax, float(QI * QI), idxf, op0=Alu.mult, op1=Alu.add)
            nc.vector.tensor_copy(idx32, idxf.rearrange("p t n -> p n t"))

        # ---------- phases D-F, tiled by pixel halves ----------
        hT = T // half_tiles                       # tiles per half
        with tc.tile_pool(name="work", bufs=3) as work, \
             tc.tile_pool(name="work2", bufs=3) as work2:
            for hf in range(half_tiles):
                t0 = hf * hT
                for n in range(N9):
                    # ---- D: gathers (2 per tap: y-corner 0 / 1) ----
                    g_t = work.tile([128, hT, 4 * CIN], bf16, tag="g")
                    if skip_gather:
                        nc.vector.memset(g_t, 0.0)
                    else:
                        for t in range(hT):
                            nc.gpsimd.indirect_dma_start(
                                out=g_t[:, t, :],
                                out_offset=None,
                                in_=xQ[:],
                                in_offset=bass.IndirectOffsetOnAxis(
                                    ap=idx32[:, n, t0 + t:t0 + t + 1], axis=0
                                ),
                                element_offset=0,
                            )
                    gv = g_t.rearrange("p t (s c) -> p t s c", s=4)

                    # ---- E: bilinear lerp (pixel-major) ----
                    # table rows hold (P00, E=P10-P00, D0=P01-P00, F=..) so
                    # xl = (P00,E) + v*(D0,F) then xoff = xl0 + u*xl1
                    xl = work2.tile([128, hT, 2, CIN], bf16, tag="xl")
                    xoffn = work2.tile([128, hT, CIN], bf16, tag="xoffn")
                    if n % 2 == 0:
                        # DVE path: fused lerp via scalar_tensor_tensor
                        for t in range(hT):
                            nc.vector.scalar_tensor_tensor(
                                xl[:, t, :, :],
                                gv[:, t, 2:4, :],
                                v_sb[:, t0 + t, n:n + 1],
                                gv[:, t, 0:2, :],
                                op0=Alu.mult,
                                op1=Alu.add,
                            )
                        for t in range(hT):
                            nc.vector.scalar_tensor_tensor(
                                xoffn[:, t, :],
                                xl[:, t, 1, :],
                                u_sb[:, t0 + t, n:n + 1],
                                xl[:, t, 0, :],
                                op0=Alu.mult,
                                op1=Alu.add,
                            )
                    else:
                        # ACT does the per-pixel-scaled mult, DVE the 2x add
                        Ident = mybir.ActivationFunctionType.Identity
                        for t in range(hT):
                            nc.scalar.activation(
                                xl[:, t, :, :],
                                gv[:, t, 2:4, :],
                                Ident,
                                scale=v_sb[:, t0 + t, n:n + 1],
                            )
                            nc.vector.tensor_tensor(
                                xl[:, t, :, :], xl[:, t, :, :],
                                gv[:, t, 0:2, :], op=Alu.add,
                            )
                        for t in range(hT):
                            nc.scalar.activation(
                                xoffn[:, t, :],
                                xl[:, t, 1, :],
                                Ident,
                                scale=u_sb[:, t0 + t, n:n + 1],
                            )
                            nc.vector.tensor_tensor(
                                xoffn[:, t, :], xoffn[:, t, :],
                                xl[:, t, 0, :], op=Alu.add,
                            )

                    # ---- F: transpose x_off back to channel-major ----
                    fg = min(8, hT)
                    for g in range(hT // fg):
                        psf = psF.tile([128, fg, 128], bf16, tag="psF")
                        for i in range(fg):
                            t = g * fg + i
                            nc.tensor.transpose(
                                psf[:, i, :], xoffn[:, t, :], identb_sb
                            )
                        dst = xoffT[
                            :, n, (t0 + g * fg) * 128:(t0 + (g + 1) * fg) * 128
                        ].rearrange("k (g c) -> k g c", c=128)
                        nc.scalar.copy(dst, psf)

                # ---- G: main conv over this half ----
                for ch in range(hT * 128 // 512):
                    c0 = t0 * 128 + ch * 512
                    ps = psG.tile([COUT, 512], f32, tag="psG")
                    for n in range(N9):
                        nc.tensor.matmul(
                            ps,
                            lhsT=wker_sb[:, n, :],
                            rhs=xoffT[:, n, c0:c0 + 512],
                            start=(n == 0),
                            stop=(n == N9 - 1),
                        )
                    nc.scalar.activation(
                        out_sb[:, c0:c0 + 512], ps,
                        mybir.ActivationFunctionType.Identity, bias=bker_sb,
                    )
                    nc.sync.dma_start(
                        out=out_d[:, c0:c0 + 512], in_=out_sb[:, c0:c0 + 512]
                    )

    nc.compile()
    return nc


# --------------------------------------------------------------------------
# host-side input prep
# --------------------------------------------------------------------------

def _prep_core_inputs(xb, w_off, b_off, w_ker, b_ker):
    """Build the per-core input map from one batch image xb (CIN, H, W)."""
    xb = np.asarray(xb, _F32)

    xP = np.zeros((CIN, H + 2, PW), _F32)
    xP[:, 1:H + 1, 1:W + 1] = xb
    xP = xP.reshape(CIN, PADHW)

    xg = np.zeros((2 * QI + 1, 2 * QI + 1, CIN), _F32)
    xg[:H, :W, :] = np.transpose(xb, (1, 2, 0))
    # parity tables: row (2a+b)*QI*QI + i*QI + j = patch (2i+a:+2, 2j+b:+2)
    P = np.zeros((4, QI, QI, 2, 2, CIN), _F32)
    for a in range(2):
        for b in range(2):
            for ry in range(2):
                for rx in range(2):
                    P[2 * a + b, :, :, ry, rx, :] = xg[
                        a + ry:a + ry + 2 * QI:2, b + rx:b + rx + 2 * QI:2, :]
    # slots: (P00, E=P10-P00, D0=P01-P00, F=P11-P10-P01+P00)
    # xl pair = (P00,E) + v*(D0,F); xoff = xl0 + u*xl1 == bilinear
    xQ = np.zeros((4, QI, QI, 4, CIN), _F32)
    xQ[..., 0, :] = P[..., 0, 0, :]
    xQ[..., 1, :] = P[..., 1, 0, :] - P[..., 0, 0, :]
    xQ[..., 2, :] = P[..., 0, 1, :] - P[..., 0, 0, :]
    xQ[..., 3, :] = P[..., 1, 1, :] - P[..., 1, 0, :] - P[..., 0, 1, :] + P[..., 0, 0, :]
    xQ = xQ.reshape(QROWS, 4 * CIN).astype(_BF16)

    woffT = np.ascontiguousarray(
        np.transpose(w_off.reshape(2 * N9, CIN, N9), (2, 1, 0))
    ).astype(_F32)                                     # [n, cin, 18]
    wkerT = np.ascontiguousarray(
        np.transpose(w_ker.reshape(COUT, CIN, N9), (2, 1, 0))
    ).astype(_BF16)                                    # [n, cin, cout]

    pix = np.arange(HW)
    hh = (pix // W).astype(_F32)
    ww = (pix % W).astype(_F32)
    pny = (np.arange(N9) // 3 - 1).astype(_F32)
    pnx = (np.arange(N9) % 3 - 1).astype(_F32)
    basey = (hh[:, None] + pny[None, :]).reshape(T, 128, N9)
    basex = (ww[:, None] + pnx[None, :]).reshape(T, 128, N9)
    basey = np.ascontiguousarray(np.transpose(basey, (1, 0, 2)))
    basex = np.ascontiguousarray(np.transpose(basex, (1, 0, 2)))

    res = {
        "xP": xP,
        "xQ": xQ,
        "woffT": woffT,
        "boff": np.asarray(b_off, _F32).reshape(2 * N9, 1),
        "wkerT": wkerT,
        "bker": np.asarray(b_ker, _F32).reshape(COUT, 1),
        "basey": basey,
        "basex": basex,
        "ident": np.eye(128, dtype=_F32),
        "identb": np.eye(128, dtype=_BF16),
    }
    return res


_PROGRAM_CACHE = {}


def kernel(x, w_off, b_off, w_ker, b_ker):
    from concourse import bass_utils

    x = np.asarray(x, _F32)
    if "nc" not in _PROGRAM_CACHE:
        _PROGRAM_CACHE["nc"] = build_program()
    nc = _PROGRAM_CACHE["nc"]

    in_maps = [
        _prep_core_inputs(x[b], np.asarray(w_off), np.asarray(b_off),
                          np.asarray(w_ker), np.asarray(b_ker))
        for b in range(B)
    ]
    res = bass_utils.run_bass_kernel_spmd(nc, in_maps, core_ids=list(range(NCORES)))
    global _LAST_RESULTS
    _LAST_RESULTS = res
    out = np.stack([res.results[b]["out"].reshape(COUT, H, W) for b in range(B)])
    return out.astype(np.float32)


_LAST_RESULTS = None
